# revision 20
# baseline (speedup 1.0000x reference)
"""Trainium2 Bass kernel for nn_CP_L3_sparse_outer.

Math (per batch row b):
    s2[b] = sum_d U2[d] * z[b, d]
    s3[b] = sum_d U3[d] * z[b, d]
    out[b, o] = (s2[b] * s3[b]) * sum_d (U1[d] * z[b, d]) * W[o, d] + bias[o]

Key identity: out = c .* ((U1 .* z) @ W.T) + bias with c = s2 * s3 a
per-batch-ROW scalar — so c is applied at PSUM eviction instead of
pre-scaling the GEMM input.

Sharding: data-parallel over batch B=8192 across 8 NeuronCores
(B_loc = 1024 rows per core); W / U1 / U2 / U3 / bias replicated.

Per-core plan (bf16 operands, f32 PSUM accumulate; bf16 matmul runs at
1 col/cycle like f32r but halves DMA/SBUF and gets fast weight loads):
  - Host prep is layout/dtype only: z.T slice per core cast bf16, W.T
    cast bf16, U1/U23 pre-tiled to the SBUF chunk layout, bias
    broadcast to 128 rows.
  - zT streams in over BOTH HWDGE queues (sync+scalar) in 8 groups,
    with the first W slab's quarters interleaved on sync — arrival
    order matches PE consumption order.
  - Per chunk k (software-pipelined, 1-chunk skew): s2/s3 matmuls
    (stationary u23 [128,2] -> psum rows [2,512]), DVE U1-fold of the
    chunk in place, then PHASE-1A main matmuls: k-major accumulation
    of oc0 x bt0..3 into 4 resident psum banks — the big GEMM starts
    while zT is still loading.
  - c: tiny PE transposes [2,128]->[128,2] of s23 + DVE mult ->
    ccol [128 b-part, bt]. Phase-1a evicts copy psum out early (bank
    release) and apply c*x+bias in a second DVE pass.
  - Remaining (oc0 x bt4..7, then oc1..7 bt-major): psum [128 b,
    512 o] accumulated over k, evicted with ONE fused DVE
    scalar_tensor_tensor: out_sb = (psum * ccol[bt]) + bias_bcast.
  - Stores on SWDGE (gpsimd), native [128 b, 512 o] tiles.
"""

import os
import sys

import numpy as np

if "/opt/trn_rl_repo" not in sys.path:
    sys.path.insert(0, "/opt/trn_rl_repo")

import concourse.bass as bass
from concourse import bacc
import concourse.mybir as mybir
import concourse.tile as tile
from concourse.masks import make_identity

P = 128
D = 4096
O = 4096
B = 8192
NCORES = 8
BLOC = B // NCORES          # 1024 batch rows per core
KC = D // P                 # 32 contraction chunks
BT = BLOC // P              # 8 batch tiles of 128
OC = O // 512               # 8 output column tiles of 512
NH = BLOC // 512            # 2 halves of the local batch
ZG = 8                      # zT DMA groups
GK = KC // ZG               # chunks per zT group
QK = 8                      # k-chunks per W quarter-slab
NQ = KC // QK               # quarter-slabs per oc
F32 = mybir.dt.float32
BF16 = mybir.dt.bfloat16
MULT = mybir.AluOpType.mult
ADD = mybir.AluOpType.add


def build_nc() -> bass.Bass:
    nc = bacc.Bacc(trn_type="TRN2")

    zt_d = nc.dram_tensor("zt", [D, BLOC], BF16, kind="ExternalInput")
    wt_d = nc.dram_tensor("wt", [D, O], BF16, kind="ExternalInput")
    u1_d = nc.dram_tensor("u1", [P, KC], F32, kind="ExternalInput")
    u23_d = nc.dram_tensor("u23", [P, KC, 2], BF16, kind="ExternalInput")
    biasb_d = nc.dram_tensor("biasb", [P, O], F32, kind="ExternalInput")
    out_d = nc.dram_tensor("out", [BLOC, O], F32, kind="ExternalOutput")

    with tile.TileContext(nc) as tc:
        with (
            tc.tile_pool(name="const", bufs=1) as const,
            tc.tile_pool(name="ztp", bufs=1) as ztp,
            tc.tile_pool(name="wslab", bufs=2 * NQ) as wslabp,
            tc.tile_pool(name="outp", bufs=9) as outp,
            tc.tile_pool(name="pmain", bufs=6, space="PSUM") as pmain,
            tc.tile_pool(name="ps23", bufs=2, space="PSUM") as ps23p,
        ):
            # ---- constants (pre-tiled on host; off the critical queues) ----
            u1sb = const.tile([P, KC], F32)
            nc.gpsimd.dma_start(u1sb[:], u1_d[:])
            u23sb = const.tile([P, KC, 2], BF16)
            nc.gpsimd.dma_start(u23sb[:], u23_d[:])
            identity = const.tile([P, P], F32)
            make_identity(nc, identity)
            s23sb = const.tile([2, BLOC], F32)
            ccol = const.tile([P, BT], F32)
            biasb = const.tile([P, O], F32)

            # zT resident: [128 d_in, k, b].  Preamble-critical bytes
            # (zT 8MB + oc0 slab 4MB) striped over all three DMA
            # dispatch queues (sync / scalar / gpsimd) so arrival
            # roughly matches PE consumption order.
            ztbig = ztp.tile([P, KC, BLOC], BF16)
            zt_view = zt_d[:].rearrange("(k p) b -> p k b", p=P)
            wt_view = wt_d[:].rearrange("(k p) o -> p k o", p=P)

            def slab_dma(eng, ws, oc, q):
                eng.dma_start(
                    ws[:],
                    wt_view[
                        :, q * QK : (q + 1) * QK, oc * 512 : (oc + 1) * 512
                    ],
                )

            # Queue scripts ordered by PE-consumption deadline: chunk k is
            # consumed ~1.7us after chunk k-1; W quarter q just before
            # phase-1a's k=8q matmul.  The first chunks go as tiny DMAs so
            # the s23 pipeline starts ASAP; s1*/bias trail the critical
            # bytes.
            wslab0 = [
                wslabp.tile([P, QK, 512], BF16, name="wslab")
                for _ in range(NQ)
            ]

            def zc_dma(eng, k0, k1):
                eng.dma_start(
                    ztbig[:, k0:k1, :], zt_view[:, k0:k1, :]
                )

            # sync:   ch0, ch1, ch2-3, ch12-15, s0c, ch24-27
            zc_dma(nc.sync, 0, 1)
            zc_dma(nc.sync, 1, 2)
            zc_dma(nc.sync, 2, 4)
            zc_dma(nc.sync, 12, 16)
            slab_dma(nc.sync, wslab0[2], 0, 2)
            zc_dma(nc.sync, 24, 28)
            # scalar: ch4-7, ch16-19, s0d, ch28-31, biasb
            zc_dma(nc.scalar, 4, 8)
            zc_dma(nc.scalar, 16, 20)
            slab_dma(nc.scalar, wslab0[3], 0, 3)
            zc_dma(nc.scalar, 28, 32)
            nc.scalar.dma_start(biasb[:], biasb_d[:])
            # gpsimd: s0a, ch8-11, s0b, ch20-23  (after the tiny consts)
            slab_dma(nc.gpsimd, wslab0[0], 0, 0)
            zc_dma(nc.gpsimd, 8, 12)
            slab_dma(nc.gpsimd, wslab0[1], 0, 1)
            zc_dma(nc.gpsimd, 20, 24)

            # ---- per-chunk pipeline: s2/s3, U1-fold, phase-1a matmuls
            # (oc0 x bt0..3, k-major into 4 resident psums; 1-chunk skew
            # so the DVE fold of chunk k hides under chunk k+1's s23) ----
            ps23 = [
                ps23p.tile([2, 512], F32, name=f"ps23_{h}", tag="s23ct")
                for h in range(NH)
            ]
            N1A = 6
            pm1a = [
                pmain.tile([P, 512], F32, name="pm", tag="pm")
                for _ in range(N1A)
            ]

            def mm1a(k):
                for bt in range(N1A):
                    nc.tensor.matmul(
                        pm1a[bt][:],
                        ztbig[:, k, bt * P : (bt + 1) * P],
                        wslab0[k // QK][:, k % QK, :],
                        start=(k == 0),
                        stop=(k == KC - 1),
                    )

            for k in range(KC):
                for h in range(NH):
                    nc.tensor.matmul(
                        ps23[h][:],
                        u23sb[:, k, :],
                        ztbig[:, k, h * 512 : (h + 1) * 512],
                        start=(k == 0),
                        stop=(k == KC - 1),
                    )
                nc.vector.tensor_scalar_mul(
                    ztbig[:, k, :], ztbig[:, k, :], u1sb[:, k : k + 1]
                )
                if k > 0:
                    mm1a(k - 1)
            mm1a(KC - 1)

            # ---- c = s2*s3 as per-partition scalars ccol [128, bt] ----
            for h in range(NH):
                nc.vector.tensor_copy(
                    s23sb[:, h * 512 : (h + 1) * 512], ps23[h][:]
                )
            ctsb = const.tile([P, BT, 2], F32)
            for bt in range(BT):
                ct = ps23p.tile([P, 2], F32, name="ct", tag="s23ct")
                nc.tensor.transpose(
                    ct[:],
                    s23sb[0:2, bt * P : (bt + 1) * P],
                    identity[0:2, 0:2],
                )
                nc.vector.tensor_copy(ctsb[:, bt, :], ct[:])
            for bt in range(BT):
                nc.vector.tensor_mul(
                    ccol[:, bt : bt + 1], ctsb[:, bt, 0:1], ctsb[:, bt, 1:2]
                )

            # ---- phase-1a evictions: raw copy frees the psum banks
            # without waiting for ccol; c*x+bias applied in place after ----
            out1a = []
            for bt in range(N1A):
                osb = outp.tile([P, 512], F32, name="outsb", tag="outsb")
                nc.vector.tensor_copy(osb[:], pm1a[bt][:])
                out1a.append(osb)
            for bt in range(N1A):
                nc.vector.scalar_tensor_tensor(
                    out1a[bt][:],
                    out1a[bt][:],
                    ccol[:, bt : bt + 1],
                    biasb[:, 0:512],
                    MULT,
                    ADD,
                )
                nc.scalar.dma_start(
                    out_d[:][bt * P : (bt + 1) * P, 0:512], out1a[bt][:]
                )

            # ---- rest of the GEMM: oc0 x bt4..7, then oc1..7 ----
            def main_tile(oc, bt, wslabs):
                psum = pmain.tile([P, 512], F32, name="pm", tag="pm")
                for k in range(KC):
                    nc.tensor.matmul(
                        psum[:],
                        ztbig[:, k, bt * P : (bt + 1) * P],
                        wslabs[k // QK][:, k % QK, :],
                        start=(k == 0),
                        stop=(k == KC - 1),
                    )
                osb = outp.tile([P, 512], F32, name="outsb", tag="outsb")
                nc.vector.scalar_tensor_tensor(
                    osb[:],
                    psum[:],
                    ccol[:, bt : bt + 1],
                    biasb[:, oc * 512 : (oc + 1) * 512],
                    MULT,
                    ADD,
                )
                nc.scalar.dma_start(
                    out_d[:][
                        bt * P : (bt + 1) * P, oc * 512 : (oc + 1) * 512
                    ],
                    osb[:],
                )

            for bt in range(N1A, BT):
                main_tile(0, bt, wslab0)
            for oc in range(1, OC):
                wslabs = []
                for q in range(NQ):
                    ws = wslabp.tile([P, QK, 512], BF16, name="wslab")
                    slab_dma(nc.sync, ws, oc, q)
                    wslabs.append(ws)
                for bt in range(BT):
                    main_tile(oc, bt, wslabs)

    nc.finalize()
    return nc


_NC_CACHE = {}


def get_nc() -> bass.Bass:
    if "nc" not in _NC_CACHE:
        _NC_CACHE["nc"] = build_nc()
    return _NC_CACHE["nc"]


def kernel(z, U1, U2, U3, W, b):
    import ml_dtypes
    from concourse.bass_utils import run_bass_kernel_spmd

    bf16 = ml_dtypes.bfloat16
    z = np.ascontiguousarray(np.asarray(z, dtype=np.float32)).reshape(B, D)
    U1 = np.asarray(U1, dtype=np.float32)
    U2 = np.asarray(U2, dtype=np.float32)
    U3 = np.asarray(U3, dtype=np.float32)
    W = np.asarray(W, dtype=np.float32)
    bias = np.asarray(b, dtype=np.float32)

    # layout/dtype-only host prep
    zb = z.astype(bf16)                                  # [B, D] bf16
    wtb = W.T.astype(bf16)                               # [D, O] bf16
    u1t = np.ascontiguousarray(U1.reshape(KC, P).T)      # [P, KC]
    u23t = np.ascontiguousarray(
        np.stack([U2, U3], 1).astype(bf16).reshape(KC, P, 2).transpose(1, 0, 2)
    )                                                    # [P, KC, 2]
    biasb = np.ascontiguousarray(
        np.broadcast_to(bias[None, :], (P, O))
    ).astype(np.float32)                                 # [128, O]

    nc = get_nc()
    in_maps = [
        {
            "zt": np.ascontiguousarray(zb[c * BLOC : (c + 1) * BLOC].T),
            "wt": wtb,
            "u1": u1t,
            "u23": u23t,
            "biasb": biasb,
        }
        for c in range(NCORES)
    ]
    res = run_bass_kernel_spmd(
        nc,
        in_maps,
        core_ids=list(range(NCORES)),
        trace=bool(int(os.environ.get("KERNEL_TRACE", "0"))),
    )
    if res.exec_time_ns is not None:
        print(f"HW exec time: {res.exec_time_ns} ns", file=sys.stderr)
    kernel.last_results = res
    return np.concatenate([res.results[c]["out"] for c in range(NCORES)], axis=0)


# revision 24
# speedup vs baseline: 1.0296x; 1.0296x over previous
"""Trainium2 Bass kernel for nn_CP_L3_sparse_outer.

Math (per batch row b):
    s2[b] = sum_d U2[d] * z[b, d]
    s3[b] = sum_d U3[d] * z[b, d]
    out[b, o] = (s2[b] * s3[b]) * sum_d (U1[d] * z[b, d]) * W[o, d] + bias[o]

Key identity: out = c .* ((U1 .* z) @ W.T) + bias with c = s2 * s3 a
per-batch-ROW scalar — so c is applied at PSUM eviction instead of
pre-scaling the GEMM input.

Sharding: data-parallel over batch B=8192 across 8 NeuronCores
(B_loc = 1024 rows per core); W / U1 / U2 / U3 / bias replicated.

Per-core plan (bf16 operands, f32 PSUM accumulate; bf16 matmul runs at
1 col/cycle like f32r but halves DMA/SBUF and gets fast weight loads):
  - Host prep is layout/dtype only: z.T slice per core cast bf16, W.T
    cast bf16, U1/U23 pre-tiled to the SBUF chunk layout, bias
    broadcast to 128 rows.
  - zT streams in over BOTH HWDGE queues (sync+scalar) in 8 groups,
    with the first W slab's quarters interleaved on sync — arrival
    order matches PE consumption order.
  - Per chunk k (software-pipelined, 1-chunk skew): s2/s3 matmuls
    (stationary u23 [128,2] -> psum rows [2,512]), DVE U1-fold of the
    chunk in place, then PHASE-1A main matmuls: k-major accumulation
    of oc0 x bt0..3 into 4 resident psum banks — the big GEMM starts
    while zT is still loading.
  - c: tiny PE transposes [2,128]->[128,2] of s23 + DVE mult ->
    ccol [128 b-part, bt]. Phase-1a evicts copy psum out early (bank
    release) and apply c*x+bias in a second DVE pass.
  - Remaining (oc0 x bt4..7, then oc1..7 bt-major): psum [128 b,
    512 o] accumulated over k, evicted with ONE fused DVE
    scalar_tensor_tensor: out_sb = (psum * ccol[bt]) + bias_bcast.
  - Stores on SWDGE (gpsimd), native [128 b, 512 o] tiles.
"""

import os
import sys

import numpy as np

if "/opt/trn_rl_repo" not in sys.path:
    sys.path.insert(0, "/opt/trn_rl_repo")

import concourse.bass as bass
from concourse import bacc
import concourse.mybir as mybir
import concourse.tile as tile
from concourse.masks import make_identity

P = 128
D = 4096
O = 4096
B = 8192
NCORES = 8
BLOC = B // NCORES          # 1024 batch rows per core
KC = D // P                 # 32 contraction chunks
BT = BLOC // P              # 8 batch tiles of 128
OC = O // 512               # 8 output column tiles of 512
NH = BLOC // 512            # 2 halves of the local batch
ZG = 8                      # zT DMA groups
GK = KC // ZG               # chunks per zT group
QK = 8                      # k-chunks per W quarter-slab
NQ = KC // QK               # quarter-slabs per oc
F32 = mybir.dt.float32
BF16 = mybir.dt.bfloat16
MULT = mybir.AluOpType.mult
ADD = mybir.AluOpType.add


def build_nc() -> bass.Bass:
    nc = bacc.Bacc(trn_type="TRN2")

    zt_d = nc.dram_tensor("zt", [D, BLOC], BF16, kind="ExternalInput")
    wt_d = nc.dram_tensor("wt", [D, O], BF16, kind="ExternalInput")
    u1_d = nc.dram_tensor("u1", [P, KC], F32, kind="ExternalInput")
    u23_d = nc.dram_tensor("u23", [P, KC, 2], BF16, kind="ExternalInput")
    biasb_d = nc.dram_tensor("biasb", [P, O], F32, kind="ExternalInput")
    out_d = nc.dram_tensor("out", [BLOC, O], F32, kind="ExternalOutput")

    with tile.TileContext(nc) as tc:
        with (
            tc.tile_pool(name="const", bufs=1) as const,
            tc.tile_pool(name="ztp", bufs=1) as ztp,
            tc.tile_pool(name="wslab", bufs=2 * NQ) as wslabp,
            tc.tile_pool(name="outp", bufs=9) as outp,
            tc.tile_pool(name="pmain", bufs=6, space="PSUM") as pmain,
            tc.tile_pool(name="ps23", bufs=2, space="PSUM") as ps23p,
        ):
            # ---- constants (pre-tiled on host; off the critical queues) ----
            u1sb = const.tile([P, KC], F32)
            nc.gpsimd.dma_start(u1sb[:], u1_d[:])
            u23sb = const.tile([P, KC, 2], BF16)
            nc.gpsimd.dma_start(u23sb[:], u23_d[:])
            identity = const.tile([P, P], F32)
            make_identity(nc, identity)
            s23sb = const.tile([2, BLOC], F32)
            ccol = const.tile([P, BT], F32)
            biasb = const.tile([P, O], F32)

            # zT resident: [128 d_in, k, b].  Preamble-critical bytes
            # (zT 8MB + oc0 slab 4MB) striped over all three DMA
            # dispatch queues (sync / scalar / gpsimd) so arrival
            # roughly matches PE consumption order.
            ztbig = ztp.tile([P, KC, BLOC], BF16)
            zt_view = zt_d[:].rearrange("(k p) b -> p k b", p=P)
            wt_view = wt_d[:].rearrange("(k p) o -> p k o", p=P)

            def slab_dma(eng, ws, oc, q):
                eng.dma_start(
                    ws[:],
                    wt_view[
                        :, q * QK : (q + 1) * QK, oc * 512 : (oc + 1) * 512
                    ],
                )

            # Queue scripts ordered by PE-consumption deadline: chunk k is
            # consumed ~1.7us after chunk k-1; W quarter q just before
            # phase-1a's k=8q matmul.  The first chunks go as tiny DMAs so
            # the s23 pipeline starts ASAP; s1*/bias trail the critical
            # bytes.
            wslab0 = [
                wslabp.tile([P, QK, 512], BF16, name="wslab")
                for _ in range(NQ)
            ]
            N1A = 6
            pm1a = [
                pmain.tile([P, 512], F32, name="pm", tag="pm")
                for _ in range(N1A)
            ]

            def zg_dma(eng, g):
                eng.dma_start(
                    ztbig[:, g * GK : (g + 1) * GK, :],
                    zt_view[:, g * GK : (g + 1) * GK, :],
                )

            # sync:   zg0, zg3, s0c, zg6
            zg_dma(nc.sync, 0)
            zg_dma(nc.sync, 3)
            slab_dma(nc.sync, wslab0[2], 0, 2)
            zg_dma(nc.sync, 6)
            # scalar: zg1, zg4, s0d, zg7, biasb
            zg_dma(nc.scalar, 1)
            zg_dma(nc.scalar, 4)
            slab_dma(nc.scalar, wslab0[3], 0, 3)
            zg_dma(nc.scalar, 7)
            nc.scalar.dma_start(biasb[:], biasb_d[:])
            # gpsimd: s0a, zg2, s0b, zg5   (after the tiny const loads)
            slab_dma(nc.gpsimd, wslab0[0], 0, 0)
            zg_dma(nc.gpsimd, 2)
            slab_dma(nc.gpsimd, wslab0[1], 0, 1)
            zg_dma(nc.gpsimd, 5)

            # HAM warmup: ~40 dependency-free matmuls on the identity so
            # the PE clock is already un-throttled (K=8/8) when the real
            # pipeline starts; pm1a[0] is cleared by its start=True MM.
            idbf = const.tile([P, P], BF16)
            nc.vector.tensor_copy(idbf[:], identity[:])
            for _ in range(40):
                nc.tensor.matmul(
                    pm1a[0][:, 0:P], idbf[:], idbf[:],
                    start=True, stop=True,
                )

            # ---- per-chunk pipeline: s2/s3, U1-fold, phase-1a matmuls
            # (oc0 x bt0..3, k-major into 4 resident psums; 1-chunk skew
            # so the DVE fold of chunk k hides under chunk k+1's s23) ----
            ps23 = [
                ps23p.tile([2, 512], F32, name=f"ps23_{h}", tag="s23ct")
                for h in range(NH)
            ]

            def mm1a(k):
                for bt in range(N1A):
                    nc.tensor.matmul(
                        pm1a[bt][:],
                        ztbig[:, k, bt * P : (bt + 1) * P],
                        wslab0[k // QK][:, k % QK, :],
                        start=(k == 0),
                        stop=(k == KC - 1),
                    )

            for k in range(KC):
                for h in range(NH):
                    nc.tensor.matmul(
                        ps23[h][:],
                        u23sb[:, k, :],
                        ztbig[:, k, h * 512 : (h + 1) * 512],
                        start=(k == 0),
                        stop=(k == KC - 1),
                    )
                nc.vector.tensor_scalar_mul(
                    ztbig[:, k, :], ztbig[:, k, :], u1sb[:, k : k + 1]
                )
                if k > 0:
                    mm1a(k - 1)
            mm1a(KC - 1)

            # ---- c = s2*s3 as per-partition scalars ccol [128, bt] ----
            for h in range(NH):
                nc.vector.tensor_copy(
                    s23sb[:, h * 512 : (h + 1) * 512], ps23[h][:]
                )
            ctsb = const.tile([P, BT, 2], F32)
            for bt in range(BT):
                ct = ps23p.tile([P, 2], F32, name="ct", tag="s23ct")
                nc.tensor.transpose(
                    ct[:],
                    s23sb[0:2, bt * P : (bt + 1) * P],
                    identity[0:2, 0:2],
                )
                nc.vector.tensor_copy(ctsb[:, bt, :], ct[:])
            for bt in range(BT):
                nc.vector.tensor_mul(
                    ccol[:, bt : bt + 1], ctsb[:, bt, 0:1], ctsb[:, bt, 1:2]
                )

            # ---- phase-1a evictions: raw copy frees the psum banks
            # without waiting for ccol; c*x+bias applied in place after ----
            out1a = []
            for bt in range(N1A):
                osb = outp.tile([P, 512], F32, name="outsb", tag="outsb")
                nc.vector.tensor_copy(osb[:], pm1a[bt][:])
                out1a.append(osb)
            for bt in range(N1A):
                nc.vector.scalar_tensor_tensor(
                    out1a[bt][:],
                    out1a[bt][:],
                    ccol[:, bt : bt + 1],
                    biasb[:, 0:512],
                    MULT,
                    ADD,
                )
                nc.scalar.dma_start(
                    out_d[:][bt * P : (bt + 1) * P, 0:512], out1a[bt][:]
                )

            # ---- rest of the GEMM: oc0 x bt4..7, then oc1..7 ----
            def main_tile(oc, bt, wslabs):
                psum = pmain.tile([P, 512], F32, name="pm", tag="pm")
                for k in range(KC):
                    nc.tensor.matmul(
                        psum[:],
                        ztbig[:, k, bt * P : (bt + 1) * P],
                        wslabs[k // QK][:, k % QK, :],
                        start=(k == 0),
                        stop=(k == KC - 1),
                    )
                osb = outp.tile([P, 512], F32, name="outsb", tag="outsb")
                nc.vector.scalar_tensor_tensor(
                    osb[:],
                    psum[:],
                    ccol[:, bt : bt + 1],
                    biasb[:, oc * 512 : (oc + 1) * 512],
                    MULT,
                    ADD,
                )
                nc.scalar.dma_start(
                    out_d[:][
                        bt * P : (bt + 1) * P, oc * 512 : (oc + 1) * 512
                    ],
                    osb[:],
                )

            for bt in range(N1A, BT):
                main_tile(0, bt, wslab0)
            for oc in range(1, OC):
                wslabs = []
                for q in range(NQ):
                    ws = wslabp.tile([P, QK, 512], BF16, name="wslab")
                    slab_dma(nc.sync, ws, oc, q)
                    wslabs.append(ws)
                for bt in range(BT):
                    main_tile(oc, bt, wslabs)

    nc.finalize()
    return nc


_NC_CACHE = {}


def get_nc() -> bass.Bass:
    if "nc" not in _NC_CACHE:
        _NC_CACHE["nc"] = build_nc()
    return _NC_CACHE["nc"]


def kernel(z, U1, U2, U3, W, b):
    import ml_dtypes
    from concourse.bass_utils import run_bass_kernel_spmd

    bf16 = ml_dtypes.bfloat16
    z = np.ascontiguousarray(np.asarray(z, dtype=np.float32)).reshape(B, D)
    U1 = np.asarray(U1, dtype=np.float32)
    U2 = np.asarray(U2, dtype=np.float32)
    U3 = np.asarray(U3, dtype=np.float32)
    W = np.asarray(W, dtype=np.float32)
    bias = np.asarray(b, dtype=np.float32)

    # layout/dtype-only host prep
    zb = z.astype(bf16)                                  # [B, D] bf16
    wtb = W.T.astype(bf16)                               # [D, O] bf16
    u1t = np.ascontiguousarray(U1.reshape(KC, P).T)      # [P, KC]
    u23t = np.ascontiguousarray(
        np.stack([U2, U3], 1).astype(bf16).reshape(KC, P, 2).transpose(1, 0, 2)
    )                                                    # [P, KC, 2]
    biasb = np.ascontiguousarray(
        np.broadcast_to(bias[None, :], (P, O))
    ).astype(np.float32)                                 # [128, O]

    nc = get_nc()
    in_maps = [
        {
            "zt": np.ascontiguousarray(zb[c * BLOC : (c + 1) * BLOC].T),
            "wt": wtb,
            "u1": u1t,
            "u23": u23t,
            "biasb": biasb,
        }
        for c in range(NCORES)
    ]
    res = run_bass_kernel_spmd(
        nc,
        in_maps,
        core_ids=list(range(NCORES)),
        trace=bool(int(os.environ.get("KERNEL_TRACE", "0"))),
    )
    if res.exec_time_ns is not None:
        print(f"HW exec time: {res.exec_time_ns} ns", file=sys.stderr)
    kernel.last_results = res
    return np.concatenate([res.results[c]["out"] for c in range(NCORES)], axis=0)
